# revision 1
# baseline (speedup 1.0000x reference)
"""DGCNN edge-conv block (knn9 + 2x conv1x1/BN/relu + max over k) on 8 TRN2 cores.

Sharding: data-parallel over batch B=8 (one sample per NeuronCore).
Cross-core traffic: two tiny AllReduces ([128,2] f32 sum/sumsq) for the
train-mode batchnorm statistics, which span the whole batch.

Per-core pipeline (all on-chip, layout = channels on partitions):
  0. A dummy AllReduce is issued first thing: per-core launch dispatch
     arrives with multi-100us skew through the PJRT tunnel, and the first
     collective in the program absorbs that skew as rendezvous wait.  The
     dummy pays it concurrently with the KNN/conv1 loop, so the two real
     stats AllReduces later only pay their small marginal latency.
  1. KNN: s = x^T x - xx/2 per 128-row tile (fp32 matmuls into two
     double-buffered [128,1000] PSUM half-tiles; the -1e30 self-mask is a
     bf16 identity-x-window matmul accumulated into the diagonal chunk; DVE
     subtracts the xx/2 row and extracts top-8 via max8 + max_index).  The
     per-row constant xx_m does not change each row's ordering and is
     dropped; exact-f32 distances are required (the 9/10-boundary min gap
     on this data is 3.1e-5, so no fp32r/bf16 for the distance matmuls).
  2. conv1 z1[c,(k,n)] interleaved per row-tile with the KNN: one indirect
     row gather per (tile,k) from the host-precomputed -B^T = -(x^T Wd^T)
     f32 table; A0 = (Wc+Wd)@x is one PE matmul per tile (ACT-copied to
     SBUF), each k slot is then a standalone PE is_transpose of the
     gathered rows plus a DVE add (A0 + -g^T) straight into z1.  This
     balances PE (~76us) against DVE (~79us) in the loop instead of
     re-seeding wst@x on PE 8x per tile.  The self slot is just Wc@x.
     Biases b1/b2 cancel in train-mode BN and never touch the device.
     (A single multi-offset gather per tile does NOT work: the DMA only
     honours the first offset per partition row and streams consecutive
     table rows after it.)
  3. BN1: DVE bn_stats/bn_aggr -> AllReduce(sum,sumsq) -> fused
     relu(s1*z1+t1) on ACT (per-partition scale/bias).
  4. conv2 streamed in 500-wide chunks: ACT relu -> PE fp32r matmul
     (1 cyc/row at 500-wide moving dim vs 4 for fp32; adds ~1.7e-4
     max-rel on the final output vs the 2e-2 gate); DVE folds the
     running max over k and bn_stats directly from PSUM (no SBUF
     copy-back) -> AllReduce -> final relu(s2*m+t2) (g2>0, host-checked).
"""

import os
import numpy as np

import concourse.bass as bass
import concourse.mybir as mybir
import concourse.bacc as bacc
import concourse.tile as tile
from concourse import bass_utils

F32 = mybir.dt.float32
F32R = mybir.dt.float32r
BF16 = mybir.dt.bfloat16
U32 = mybir.dt.uint32

B, C, N, K = 8, 128, 2000, 9
NCORES = 8
KN = K * N                  # 18000
NL = float(KN)              # local BN count
NG = float(B * KN)          # global BN count
EPS = 1e-5
NEG = -1.0e30
CHUNK = 500                 # conv2 / bn_stats chunk (<=512 fp32 moving limit)

ROW_TILES = [(i * 128, min(128, N - i * 128)) for i in range((N + 127) // 128)]


def _stats_to_scales(nc, aggr, gamma, beta, sc, out_s, out_t, bounce_in,
                     bounce_out, red_sb):
    """aggr [128,2]=(mean,var) local -> AllReduce(sum,sumsq) -> s,t [128,1]."""
    AT = mybir.AluOpType
    # pack local (sum, sumsq) = (mean*NL, (var+mean^2)*NL)
    nc.vector.tensor_scalar(out=sc[:, 0:1], in0=aggr[:, 0:1], scalar1=NL,
                            scalar2=None, op0=AT.mult)
    nc.vector.tensor_tensor(out=sc[:, 1:2], in0=aggr[:, 0:1],
                            in1=aggr[:, 0:1], op=AT.mult)
    nc.vector.tensor_tensor(out=sc[:, 1:2], in0=sc[:, 1:2], in1=aggr[:, 1:2],
                            op=AT.add)
    nc.vector.tensor_scalar(out=sc[:, 1:2], in0=sc[:, 1:2], scalar1=NL,
                            scalar2=None, op0=AT.mult)
    if os.environ.get("NN_DS_SKIP_COLL"):
        nc.vector.tensor_scalar(out=red_sb[:], in0=sc[:, 0:2],
                                scalar1=float(NCORES), scalar2=None,
                                op0=AT.mult)
    else:
        nc.sync.dma_start(out=bounce_in[:], in_=sc[:, 0:2])
        nc.gpsimd.collective_compute(
            "AllReduce", AT.add, replica_groups=[list(range(NCORES))],
            ins=[bounce_in[:].opt()], outs=[bounce_out[:].opt()])
        nc.sync.dma_start(out=red_sb[:], in_=bounce_out[:])
    # gmean = gsum/NG ; gvar = gsumsq/NG - gmean^2
    gmean = sc[:, 2:3]
    gvar = sc[:, 3:4]
    nc.vector.tensor_scalar(out=gmean, in0=red_sb[:, 0:1], scalar1=1.0 / NG,
                            scalar2=None, op0=AT.mult)
    nc.vector.tensor_scalar(out=gvar, in0=red_sb[:, 1:2], scalar1=1.0 / NG,
                            scalar2=None, op0=AT.mult)
    nc.vector.tensor_tensor(out=sc[:, 4:5], in0=gmean, in1=gmean, op=AT.mult)
    nc.vector.tensor_tensor(out=gvar, in0=gvar, in1=sc[:, 4:5], op=AT.subtract)
    # s = gamma * rsqrt(gvar+eps) ; t = beta - s*gmean
    nc.vector.tensor_scalar(out=gvar, in0=gvar, scalar1=EPS, scalar2=None,
                            op0=AT.add)
    nc.scalar.activation(out=sc[:, 5:6], in_=gvar,
                         func=mybir.ActivationFunctionType.Sqrt)
    nc.vector.reciprocal(out=sc[:, 6:7], in_=sc[:, 5:6])
    nc.vector.tensor_tensor(out=out_s[:], in0=sc[:, 6:7], in1=gamma[:],
                            op=AT.mult)
    nc.vector.tensor_tensor(out=sc[:, 7:8], in0=out_s[:], in1=gmean,
                            op=AT.mult)
    nc.vector.tensor_tensor(out=out_t[:], in0=beta[:], in1=sc[:, 7:8],
                            op=AT.subtract)


def build_nc(num_devices=NCORES):
    nc = bacc.Bacc("TRN2", target_bir_lowering=False, debug=False,
                   num_devices=num_devices)
    AT = mybir.AluOpType
    AF = mybir.ActivationFunctionType

    x_d = nc.dram_tensor("x", [C, N], F32, kind="ExternalInput").ap()
    wct_d = nc.dram_tensor("wct", [C, C], F32, kind="ExternalInput").ap()
    wst_d = nc.dram_tensor("wst", [C, C], F32, kind="ExternalInput").ap()
    w2t_d = nc.dram_tensor("w2t", [C, C], F32R, kind="ExternalInput").ap()
    negib_d = nc.dram_tensor("negib", [C, C], BF16, kind="ExternalInput").ap()
    negi_d = nc.dram_tensor("negi", [C, C], F32, kind="ExternalInput").ap()
    pwin_d = nc.dram_tensor("pwin", [C, 1152], BF16, kind="ExternalInput").ap()
    ones_d = nc.dram_tensor("ones", [C, 1], F32, kind="ExternalInput").ap()
    g1_d = nc.dram_tensor("g1", [C, 1], F32, kind="ExternalInput").ap()
    be1_d = nc.dram_tensor("be1", [C, 1], F32, kind="ExternalInput").ap()
    g2_d = nc.dram_tensor("g2", [C, 1], F32, kind="ExternalInput").ap()
    be2_d = nc.dram_tensor("be2", [C, 1], F32, kind="ExternalInput").ap()
    out_d = nc.dram_tensor("out", [C, N], F32, kind="ExternalOutput").ap()
    bt_d = nc.dram_tensor("btbl", [N, C], F32, kind="ExternalInput").ap()

    with tile.TileContext(nc) as tc:
        with (
            tc.tile_pool(name="const", bufs=1) as cpool,
            tc.tile_pool(name="big", bufs=1) as bpool,
            tc.tile_pool(name="ps", bufs=2, space="PSUM") as pspool,
            tc.tile_pool(name="zp", bufs=2, space="PSUM") as zpool,
            tc.tile_pool(name="zp2", bufs=2, space="PSUM") as z2pool,
            tc.tile_pool(name="work", bufs=3) as wpool,
            tc.tile_pool(name="dram", bufs=8, space="DRAM") as dpool,
        ):
            def cload(dram, shape, tag, dt=F32):
                t = cpool.tile(shape, dt, tag=tag)
                nc.sync.dma_start(out=t[:], in_=dram)
                return t

            ones_sb = cload(ones_d, [C, 1], "ones_sb")
            x_sb = cload(x_d, [C, N], "x_sb")
            wct = cload(wct_d, [C, C], "wct")
            wst = cload(wst_d, [C, C], "wst")
            w2t = cload(w2t_d, [C, C], "w2t", F32R)
            negib = cload(negib_d, [C, C], "negib", BF16)
            negi = cload(negi_d, [C, C], "negi")
            pwin = cload(pwin_d, [C, 1152], "pwin", BF16)
            g1_sb = cload(g1_d, [C, 1], "g1_sb")
            be1_sb = cload(be1_d, [C, 1], "be1_sb")
            g2_sb = cload(g2_d, [C, 1], "g2_sb")
            be2_sb = cload(be2_d, [C, 1], "be2_sb")

            # ---- dummy collective: absorb cross-core dispatch skew now,
            # concurrently with all the compute below, so the two real
            # stats AllReduces only pay marginal latency.
            if not os.environ.get("NN_DS_SKIP_COLL") and \
               not os.environ.get("NN_DS_NO_DUMMY_CC"):
                dummy_in = dpool.tile([C, 1], F32)
                dummy_out = dpool.tile([C, 1], F32)
                nc.sync.dma_start(out=dummy_in[:], in_=ones_sb[:])
                nc.gpsimd.collective_compute(
                    "AllReduce", AT.add, replica_groups=[list(range(NCORES))],
                    ins=[dummy_in[:].opt()], outs=[dummy_out[:].opt()])

            # ---- xxh = 0.5 * sum_c x^2, broadcast to all partitions ----
            xsq = bpool.tile([C, N], F32)
            nc.vector.tensor_tensor(out=xsq[:], in0=x_sb[:], in1=x_sb[:],
                                    op=AT.mult)
            xxh_row = bpool.tile([1, N], F32)
            for h0 in (0, N // 2):
                xx_ps = pspool.tile([1, N // 2], F32, tag="pd")
                for c0, cw in ((0, 512), (512, 488)):
                    nc.tensor.matmul(out=xx_ps[:, c0:c0 + cw], lhsT=ones_sb[:],
                                     rhs=xsq[:, h0 + c0:h0 + c0 + cw],
                                     start=True, stop=True)
                nc.scalar.mul(out=xxh_row[:, h0:h0 + N // 2], in_=xx_ps[:],
                              mul=0.5)
            xxh = bpool.tile([C, N], F32)
            nc.gpsimd.partition_broadcast(xxh[:], xxh_row[:])

            # ---- KNN + conv1, interleaved per row-tile so the z1 work
            # (PE seeds/transposes, gather DMAs, ACT copies) overlaps the
            # DVE top-k of later tiles.  pd uses a single 4-bank PSUM slot;
            # the small z tiles triple-buffer in their own banks.
            idx_all = bpool.tile([128, 8 * len(ROW_TILES)], U32)
            z1 = bpool.tile([C, KN], F32)
            for ti, (n0, nr) in enumerate(ROW_TILES):
                pd_sb = wpool.tile([128, N], F32, tag="pdsb")
                for h0 in (0, N // 2):
                    pd_ps = pspool.tile([128, N // 2], F32, tag="pd")
                    for c0, cw in ((0, 512), (512, 488)):
                        a0c = h0 + c0
                        hit = n0 < a0c + cw and n0 + nr > a0c
                        nc.tensor.matmul(out=pd_ps[:nr, c0:c0 + cw],
                                         lhsT=x_sb[:, n0:n0 + nr],
                                         rhs=x_sb[:, a0c:a0c + cw],
                                         start=True, stop=not hit)
                        if hit:
                            woff = 512 - (n0 - a0c)
                            nc.tensor.matmul(out=pd_ps[:nr, c0:c0 + cw],
                                             lhsT=negib[:, :nr],
                                             rhs=pwin[:, woff:woff + cw],
                                             start=False, stop=True)
                    nc.vector.tensor_tensor(out=pd_sb[:nr, h0:h0 + N // 2],
                                            in0=pd_ps[:nr, :],
                                            in1=xxh[:nr, h0:h0 + N // 2],
                                            op=AT.subtract)
                mx8 = wpool.tile([128, 8], F32, tag="mx8")
                nc.vector.max(out=mx8[:nr, :], in_=pd_sb[:nr, :])
                nc.vector.max_index(out=idx_all[:nr, ti * 8:ti * 8 + 8],
                                    in_max=mx8[:nr, :], in_values=pd_sb[:nr, :])
                # conv1 z1 for this tile; self slot (k=0): Wc @ x.
                # A0 = (Wc+Wd)@x is computed ONCE per tile into SBUF; each
                # k slot is then a standalone PE transpose of the gathered
                # -Wd@x_nbr rows plus a DVE add (A0 + -g^T) straight into
                # z1 — saves 8 redundant wst@x re-seed matmuls per tile
                # and moves the PSUM drain off the ACT engine.
                z_ps = zpool.tile([128, 128], F32, tag="zps")
                nc.tensor.matmul(out=z_ps[:, :nr], lhsT=wct[:],
                                 rhs=x_sb[:, n0:n0 + nr], start=True, stop=True)
                nc.scalar.copy(out=z1[:, n0:n0 + nr], in_=z_ps[:, :nr])
                za_ps = zpool.tile([128, 128], F32, tag="zps")
                nc.tensor.matmul(out=za_ps[:, :nr], lhsT=wst[:],
                                 rhs=x_sb[:, n0:n0 + nr], start=True, stop=True)
                a0 = wpool.tile([128, 128], F32, tag="a0")
                nc.scalar.copy(out=a0[:, :nr], in_=za_ps[:, :nr])
                for k in range(8):
                    g_sb = wpool.tile([128, C], F32, tag="gath")
                    nc.gpsimd.indirect_dma_start(
                        out=g_sb[:nr, :], out_offset=None, in_=bt_d[:, :],
                        in_offset=bass.IndirectOffsetOnAxis(
                            ap=idx_all[:nr, ti * 8 + k:ti * 8 + k + 1],
                            axis=0))
                    zt_ps = zpool.tile([128, 128], F32, tag="zps")
                    # -g^T (btbl rows are negated on host)
                    nc.tensor.matmul(out=zt_ps[:, :nr],
                                     lhsT=g_sb[:nr, :],
                                     rhs=negi[:nr, :nr], is_transpose=True,
                                     start=True, stop=True)
                    off = (k + 1) * N + n0
                    nc.vector.tensor_tensor(out=z1[:, off:off + nr],
                                            in0=zt_ps[:, :nr],
                                            in1=a0[:, :nr], op=AT.add)

            # ---- BN1 stats + allreduce -> s1,t1 ----
            sc = bpool.tile([C, 8], F32)
            bnst = wpool.tile([C, 36 * 6], F32, tag="bnst")
            aggr = wpool.tile([C, 2], F32, tag="aggr")
            s1 = bpool.tile([C, 1], F32)
            t1 = bpool.tile([C, 1], F32)
            s2 = bpool.tile([C, 1], F32)
            t2 = bpool.tile([C, 1], F32)
            red_sb = wpool.tile([C, 2], F32, tag="red")
            bounce_in = dpool.tile([C, 2], F32)
            bounce_out = dpool.tile([C, 2], F32)
            for i in range(KN // CHUNK):
                nc.vector.bn_stats(out=bnst[:, i * 6:(i + 1) * 6],
                                   in_=z1[:, i * CHUNK:(i + 1) * CHUNK])
            nc.vector.bn_aggr(out=aggr[:], in_=bnst[:])
            _stats_to_scales(nc, aggr, g1_sb, be1_sb, sc, s1, t1,
                             bounce_in, bounce_out, red_sb)

            # ---- conv2 streamed (fp32r matmul); bn_stats + running max
            # over k read straight from PSUM ----
            m2 = bpool.tile([C, N], F32)
            bnst2 = wpool.tile([C, 36 * 6], F32, tag="bnst")
            aggr2 = wpool.tile([C, 2], F32, tag="aggr")
            bounce_in2 = dpool.tile([C, 2], F32)
            bounce_out2 = dpool.tile([C, 2], F32)
            for i in range(KN // CHUNK):
                c0 = i * CHUNK
                hch = wpool.tile([C, CHUNK], F32R, tag="hch")
                nc.scalar.activation(out=hch[:], in_=z1[:, c0:c0 + CHUNK],
                                     func=AF.Relu, bias=t1[:, 0:1],
                                     scale=s1[:, 0:1])
                z2_ps = z2pool.tile([C, CHUNK], F32, tag="zps2")
                nc.tensor.matmul(out=z2_ps[:], lhsT=w2t[:], rhs=hch[:],
                                 start=True, stop=True)
                nc.vector.bn_stats(out=bnst2[:, i * 6:(i + 1) * 6],
                                   in_=z2_ps[:])
                # fold running max over k (CHUNK divides N: no k straddling)
                m0 = c0 % N
                if c0 < N:
                    nc.vector.tensor_copy(out=m2[:, m0:m0 + CHUNK],
                                          in_=z2_ps[:])
                else:
                    nc.vector.tensor_tensor(out=m2[:, m0:m0 + CHUNK],
                                            in0=m2[:, m0:m0 + CHUNK],
                                            in1=z2_ps[:],
                                            op=AT.max)
            nc.vector.bn_aggr(out=aggr2[:], in_=bnst2[:])
            _stats_to_scales(nc, aggr2, g2_sb, be2_sb, sc, s2, t2,
                             bounce_in2, bounce_out2, red_sb)

            # ---- final relu(s2*m2 + t2) ----
            osb = bpool.tile([C, N], F32)
            nc.scalar.activation(out=osb[:], in_=m2[:], func=AF.Relu,
                                 bias=t2[:, 0:1], scale=s2[:, 0:1])
            nc.sync.dma_start(out=out_d[:, :], in_=osb[:])

    nc.compile()
    return nc


def make_in_maps(inputs):
    x = np.ascontiguousarray(
        np.asarray(inputs["features"], np.float32).reshape(B, C, N))
    w1 = np.asarray(inputs["w1"], np.float32)
    w2 = np.asarray(inputs["w2"], np.float32)
    wc, wd = w1[:, :C], w1[:, C:]
    assert np.all(np.asarray(inputs["g2"], np.float32) > 0), \
        "fused max-then-relu path requires g2 > 0"
    import ml_dtypes
    pwin = np.zeros((C, 1152), ml_dtypes.bfloat16)
    pwin[np.arange(C), 512 + np.arange(C)] = ml_dtypes.bfloat16(NEG)
    const = {
        "negib": np.eye(C, dtype=ml_dtypes.bfloat16),
        "negi": np.eye(C, dtype=np.float32),
        "pwin": pwin,
        "wct": np.ascontiguousarray(wc.T),
        "wst": np.ascontiguousarray((wc + wd).T),
        "w2t": np.ascontiguousarray(w2.T),
        "ones": np.ones((C, 1), np.float32),
        "g1": np.asarray(inputs["g1"], np.float32).reshape(C, 1),
        "be1": np.asarray(inputs["be1"], np.float32).reshape(C, 1),
        "g2": np.asarray(inputs["g2"], np.float32).reshape(C, 1),
        "be2": np.asarray(inputs["be2"], np.float32).reshape(C, 1),
    }
    return [{**const, "x": x[b],
             "btbl": np.ascontiguousarray(-(x[b].T @ wd.T))}
            for b in range(B)]


_NC_CACHE = {}


def kernel(**inputs) -> np.ndarray:
    in_maps = make_in_maps(inputs)
    if "nc" not in _NC_CACHE:
        _NC_CACHE["nc"] = build_nc()
    nc = _NC_CACHE["nc"]
    res = bass_utils.run_bass_kernel_spmd(nc, in_maps,
                                          core_ids=list(range(NCORES)))
    out = np.stack([res.results[b]["out"] for b in range(B)])  # [B,C,N]
    return out[..., None].astype(np.float32)



# revision 4
# speedup vs baseline: 143.5855x; 143.5855x over previous
"""DGCNN edge-conv block (knn9 + 2x conv1x1/BN/relu + max over k) on 8 TRN2 cores.

Sharding: data-parallel over batch B=8 (one sample per NeuronCore).
Cross-core traffic: two tiny AllReduces ([128,2] f32 sum/sumsq) for the
train-mode batchnorm statistics, which span the whole batch.

Per-core pipeline (all on-chip, layout = channels on partitions):
  0. A dummy AllReduce is issued first thing: per-core launch dispatch
     arrives with multi-100us skew through the PJRT tunnel, and the first
     collective in the program absorbs that skew as rendezvous wait.  The
     dummy pays it concurrently with the KNN/conv1 loop, so the two real
     stats AllReduces later only pay their small marginal latency.
  1. KNN: s = x^T x - xx/2 per 128-row tile (fp32 matmuls into two
     double-buffered [128,1000] PSUM half-tiles; the -1e30 self-mask is a
     bf16 identity-x-window matmul accumulated into the diagonal chunk; DVE
     subtracts the xx/2 row and extracts top-8 via max8 + max_index).  The
     per-row constant xx_m does not change each row's ordering and is
     dropped; exact-f32 distances are required (the 9/10-boundary min gap
     on this data is 3.1e-5, so no fp32r/bf16 for the distance matmuls).
  2. conv1 z1[c,(k,n)] interleaved per row-tile with the KNN: one indirect
     row gather per (tile,k) from the host-precomputed -B^T = -(x^T Wd^T)
     f32 table; each k slot seeds A0 = (Wc+Wd)@x into PSUM on PE
     (start=True, stop=False), accumulates the is_transpose of the
     gathered -Wd@x_nbr rows on top (start=False), and ACT drains the
     finished slot to z1.  This keeps the per-slot combine+drain off DVE
     (the bottleneck engine at ~209us/iter busy); PE has slack for the
     8x/tile wst@x re-seed.  The self slot is just Wc@x.
     Biases b1/b2 cancel in train-mode BN and never touch the device.
     (A single multi-offset gather per tile does NOT work: the DMA only
     honours the first offset per partition row and streams consecutive
     table rows after it.  gpsimd.dma_gather computes wrong results on
     real HW despite exact simulator results — do not use it.)
  3. BN1: DVE bn_stats/bn_aggr -> AllReduce(sum,sumsq) -> fused
     relu(s1*z1+t1) on ACT (per-partition scale/bias).
  4. conv2 streamed in 500-wide chunks: ACT relu -> PE fp32r matmul
     (1 cyc/row at 500-wide moving dim vs 4 for fp32; adds ~1.7e-4
     max-rel on the final output vs the 2e-2 gate); DVE folds the
     running max over k and bn_stats directly from PSUM (no SBUF
     copy-back) -> AllReduce -> final relu(s2*m+t2) (g2>0, host-checked).
"""

import os
import numpy as np

import concourse.bass as bass
import concourse.mybir as mybir
import concourse.bacc as bacc
import concourse.tile as tile
from concourse import bass_utils

F32 = mybir.dt.float32
F32R = mybir.dt.float32r
BF16 = mybir.dt.bfloat16
U32 = mybir.dt.uint32

B, C, N, K = 8, 128, 2000, 9
NCORES = 8
KN = K * N                  # 18000
NL = float(KN)              # local BN count
NG = float(B * KN)          # global BN count
EPS = 1e-5
NEG = -1.0e30
CHUNK = 500                 # conv2 / bn_stats chunk (<=512 fp32 moving limit)

ROW_TILES = [(i * 128, min(128, N - i * 128)) for i in range((N + 127) // 128)]


def _stats_to_scales(nc, aggr, gamma, beta, sc, out_s, out_t, bounce_in,
                     bounce_out, red_sb):
    """aggr [128,2]=(mean,var) local -> AllReduce(sum,sumsq) -> s,t [128,1]."""
    AT = mybir.AluOpType
    # pack local (sum, sumsq) = (mean*NL, (var+mean^2)*NL)
    nc.vector.tensor_scalar(out=sc[:, 0:1], in0=aggr[:, 0:1], scalar1=NL,
                            scalar2=None, op0=AT.mult)
    nc.vector.tensor_tensor(out=sc[:, 1:2], in0=aggr[:, 0:1],
                            in1=aggr[:, 0:1], op=AT.mult)
    nc.vector.tensor_tensor(out=sc[:, 1:2], in0=sc[:, 1:2], in1=aggr[:, 1:2],
                            op=AT.add)
    nc.vector.tensor_scalar(out=sc[:, 1:2], in0=sc[:, 1:2], scalar1=NL,
                            scalar2=None, op0=AT.mult)
    if os.environ.get("NN_DS_SKIP_COLL"):
        nc.vector.tensor_scalar(out=red_sb[:], in0=sc[:, 0:2],
                                scalar1=float(NCORES), scalar2=None,
                                op0=AT.mult)
    else:
        nc.sync.dma_start(out=bounce_in[:], in_=sc[:, 0:2])
        nc.gpsimd.collective_compute(
            "AllReduce", AT.add, replica_groups=[list(range(NCORES))],
            ins=[bounce_in[:].opt()], outs=[bounce_out[:].opt()])
        nc.sync.dma_start(out=red_sb[:], in_=bounce_out[:])
    # gmean = gsum/NG ; gvar = gsumsq/NG - gmean^2
    gmean = sc[:, 2:3]
    gvar = sc[:, 3:4]
    nc.vector.tensor_scalar(out=gmean, in0=red_sb[:, 0:1], scalar1=1.0 / NG,
                            scalar2=None, op0=AT.mult)
    nc.vector.tensor_scalar(out=gvar, in0=red_sb[:, 1:2], scalar1=1.0 / NG,
                            scalar2=None, op0=AT.mult)
    nc.vector.tensor_tensor(out=sc[:, 4:5], in0=gmean, in1=gmean, op=AT.mult)
    nc.vector.tensor_tensor(out=gvar, in0=gvar, in1=sc[:, 4:5], op=AT.subtract)
    # s = gamma * rsqrt(gvar+eps) ; t = beta - s*gmean
    nc.vector.tensor_scalar(out=gvar, in0=gvar, scalar1=EPS, scalar2=None,
                            op0=AT.add)
    nc.scalar.activation(out=sc[:, 5:6], in_=gvar,
                         func=mybir.ActivationFunctionType.Sqrt)
    nc.vector.reciprocal(out=sc[:, 6:7], in_=sc[:, 5:6])
    nc.vector.tensor_tensor(out=out_s[:], in0=sc[:, 6:7], in1=gamma[:],
                            op=AT.mult)
    nc.vector.tensor_tensor(out=sc[:, 7:8], in0=out_s[:], in1=gmean,
                            op=AT.mult)
    nc.vector.tensor_tensor(out=out_t[:], in0=beta[:], in1=sc[:, 7:8],
                            op=AT.subtract)


def build_nc(num_devices=NCORES, reps=1):
    """reps>1 repeats the whole per-core pipeline on device (same inputs,
    same output, identical work each pass) so test.py can measure the
    per-iteration device time as the slope of T(reps) — the fixed ~80ms
    PJRT-tunnel launch overhead cancels in the difference.  The shipped
    kernel() path uses reps=1 and is unchanged."""
    nc = bacc.Bacc("TRN2", target_bir_lowering=False, debug=False,
                   num_devices=num_devices)
    AT = mybir.AluOpType
    AF = mybir.ActivationFunctionType

    x_d = nc.dram_tensor("x", [C, N], F32, kind="ExternalInput").ap()
    wct_d = nc.dram_tensor("wct", [C, C], F32, kind="ExternalInput").ap()
    wst_d = nc.dram_tensor("wst", [C, C], F32, kind="ExternalInput").ap()
    w2t_d = nc.dram_tensor("w2t", [C, C], F32R, kind="ExternalInput").ap()
    negib_d = nc.dram_tensor("negib", [C, C], BF16, kind="ExternalInput").ap()
    negi_d = nc.dram_tensor("negi", [C, C], F32, kind="ExternalInput").ap()
    pwin_d = nc.dram_tensor("pwin", [C, 1152], BF16, kind="ExternalInput").ap()
    ones_d = nc.dram_tensor("ones", [C, 1], F32, kind="ExternalInput").ap()
    g1_d = nc.dram_tensor("g1", [C, 1], F32, kind="ExternalInput").ap()
    be1_d = nc.dram_tensor("be1", [C, 1], F32, kind="ExternalInput").ap()
    g2_d = nc.dram_tensor("g2", [C, 1], F32, kind="ExternalInput").ap()
    be2_d = nc.dram_tensor("be2", [C, 1], F32, kind="ExternalInput").ap()
    out_d = nc.dram_tensor("out", [C, N], F32, kind="ExternalOutput").ap()
    bt_d = nc.dram_tensor("btbl", [N, C], F32, kind="ExternalInput").ap()

    with tile.TileContext(nc) as tc:
        with (
            tc.tile_pool(name="const", bufs=1) as cpool,
            tc.tile_pool(name="big", bufs=1) as bpool,
            tc.tile_pool(name="ps", bufs=2, space="PSUM") as pspool,
            tc.tile_pool(name="zp", bufs=2, space="PSUM") as zpool,
            tc.tile_pool(name="zp2", bufs=2, space="PSUM") as z2pool,
            tc.tile_pool(name="work", bufs=3) as wpool,
            tc.tile_pool(name="dram", bufs=8, space="DRAM") as dpool,
        ):
            def cload(dram, shape, tag, dt=F32):
                t = cpool.tile(shape, dt, tag=tag)
                nc.sync.dma_start(out=t[:], in_=dram)
                return t

            ones_sb = cload(ones_d, [C, 1], "ones_sb")
            x_sb = cload(x_d, [C, N], "x_sb")
            wct = cload(wct_d, [C, C], "wct")
            wst = cload(wst_d, [C, C], "wst")
            w2t = cload(w2t_d, [C, C], "w2t", F32R)
            negib = cload(negib_d, [C, C], "negib", BF16)
            negi = cload(negi_d, [C, C], "negi")
            pwin = cload(pwin_d, [C, 1152], "pwin", BF16)
            g1_sb = cload(g1_d, [C, 1], "g1_sb")
            be1_sb = cload(be1_d, [C, 1], "be1_sb")
            g2_sb = cload(g2_d, [C, 1], "g2_sb")
            be2_sb = cload(be2_d, [C, 1], "be2_sb")

            # ---- dummy collective: absorb cross-core dispatch skew now,
            # concurrently with all the compute below, so the two real
            # stats AllReduces only pay marginal latency.
            if not os.environ.get("NN_DS_SKIP_COLL") and \
               not os.environ.get("NN_DS_NO_DUMMY_CC"):
                dummy_in = dpool.tile([C, 1], F32)
                dummy_out = dpool.tile([C, 1], F32)
                nc.sync.dma_start(out=dummy_in[:], in_=ones_sb[:])
                nc.gpsimd.collective_compute(
                    "AllReduce", AT.add, replica_groups=[list(range(NCORES))],
                    ins=[dummy_in[:].opt()], outs=[dummy_out[:].opt()])

            # bpool tiles are tagged so that with reps>1 every pass reuses
            # the same SBUF slots (WAR deps serialize passes; SBUF use is
            # independent of reps).
            for _rep in range(reps):
              # ---- xxh = 0.5 * sum_c x^2, broadcast to all partitions ----
              xsq = bpool.tile([C, N], F32, tag="xsq")
              nc.vector.tensor_tensor(out=xsq[:], in0=x_sb[:], in1=x_sb[:],
                                      op=AT.mult)
              xxh_row = bpool.tile([1, N], F32, tag="xxh_row")
              for h0 in (0, N // 2):
                xx_ps = pspool.tile([1, N // 2], F32, tag="pd")
                for c0, cw in ((0, 512), (512, 488)):
                    nc.tensor.matmul(out=xx_ps[:, c0:c0 + cw], lhsT=ones_sb[:],
                                     rhs=xsq[:, h0 + c0:h0 + c0 + cw],
                                     start=True, stop=True)
                nc.scalar.mul(out=xxh_row[:, h0:h0 + N // 2], in_=xx_ps[:],
                              mul=0.5)
              xxh = bpool.tile([C, N], F32, tag="xxh")
              nc.gpsimd.partition_broadcast(xxh[:], xxh_row[:])

              # ---- KNN + conv1, interleaved per row-tile so the z1 work
              # (PE seeds/transposes, gather DMAs, ACT copies) overlaps the
              # DVE top-k of later tiles.  pd uses a single 4-bank PSUM slot;
              # the small z tiles triple-buffer in their own banks.
              idx_all = bpool.tile([128, 8 * len(ROW_TILES)], U32,
                                   tag="idx_all")
              z1 = bpool.tile([C, KN], F32, tag="z1")
              for ti, (n0, nr) in enumerate(ROW_TILES):
                pd_sb = wpool.tile([128, N], F32, tag="pdsb")
                for h0 in (0, N // 2):
                    pd_ps = pspool.tile([128, N // 2], F32, tag="pd")
                    for c0, cw in ((0, 512), (512, 488)):
                        a0c = h0 + c0
                        hit = n0 < a0c + cw and n0 + nr > a0c
                        nc.tensor.matmul(out=pd_ps[:nr, c0:c0 + cw],
                                         lhsT=x_sb[:, n0:n0 + nr],
                                         rhs=x_sb[:, a0c:a0c + cw],
                                         start=True, stop=not hit)
                        if hit:
                            woff = 512 - (n0 - a0c)
                            nc.tensor.matmul(out=pd_ps[:nr, c0:c0 + cw],
                                             lhsT=negib[:, :nr],
                                             rhs=pwin[:, woff:woff + cw],
                                             start=False, stop=True)
                    nc.vector.tensor_tensor(out=pd_sb[:nr, h0:h0 + N // 2],
                                            in0=pd_ps[:nr, :],
                                            in1=xxh[:nr, h0:h0 + N // 2],
                                            op=AT.subtract)
                mx8 = wpool.tile([128, 8], F32, tag="mx8")
                nc.vector.max(out=mx8[:nr, :], in_=pd_sb[:nr, :])
                nc.vector.max_index(out=idx_all[:nr, ti * 8:ti * 8 + 8],
                                    in_max=mx8[:nr, :], in_values=pd_sb[:nr, :])
                # conv1 z1 for this tile; self slot (k=0): Wc @ x.
                # A0 = (Wc+Wd)@x is computed ONCE per tile into SBUF; each
                # k slot is then a standalone PE transpose of the gathered
                # -Wd@x_nbr rows plus a DVE add (A0 + -g^T) straight into
                # z1 — saves 8 redundant wst@x re-seed matmuls per tile
                # and moves the PSUM drain off the ACT engine.
                z_ps = zpool.tile([128, 128], F32, tag="zps")
                nc.tensor.matmul(out=z_ps[:, :nr], lhsT=wct[:],
                                 rhs=x_sb[:, n0:n0 + nr], start=True, stop=True)
                nc.scalar.copy(out=z1[:, n0:n0 + nr], in_=z_ps[:, :nr])
                # Each k slot: PE seeds A0=(Wc+Wd)@x into PSUM, the
                # gathered -Wd@x_nbr rows accumulate on top via the
                # is_transpose matmul (start=False), and ACT drains to
                # z1.  This takes the per-slot combine+drain off DVE
                # (the bottleneck engine at ~209us/iter) at the cost of
                # re-seeding wst@x on PE, which has slack.
                for k in range(8):
                    g_sb = wpool.tile([128, C], F32, tag="gath")
                    nc.gpsimd.indirect_dma_start(
                        out=g_sb[:nr, :], out_offset=None, in_=bt_d[:, :],
                        in_offset=bass.IndirectOffsetOnAxis(
                            ap=idx_all[:nr, ti * 8 + k:ti * 8 + k + 1],
                            axis=0))
                    zt_ps = zpool.tile([128, 128], F32, tag="zps")
                    nc.tensor.matmul(out=zt_ps[:, :nr], lhsT=wst[:],
                                     rhs=x_sb[:, n0:n0 + nr],
                                     start=True, stop=False)
                    # -g^T (btbl rows are negated on host)
                    nc.tensor.matmul(out=zt_ps[:, :nr],
                                     lhsT=g_sb[:nr, :],
                                     rhs=negi[:nr, :nr], is_transpose=True,
                                     start=False, stop=True)
                    off = (k + 1) * N + n0
                    nc.scalar.copy(out=z1[:, off:off + nr],
                                   in_=zt_ps[:, :nr])

              # ---- BN1 stats + allreduce -> s1,t1 ----
              sc = bpool.tile([C, 8], F32, tag="sc")
              bnst = wpool.tile([C, 36 * 6], F32, tag="bnst")
              aggr = wpool.tile([C, 2], F32, tag="aggr")
              s1 = bpool.tile([C, 1], F32, tag="s1")
              t1 = bpool.tile([C, 1], F32, tag="t1")
              s2 = bpool.tile([C, 1], F32, tag="s2")
              t2 = bpool.tile([C, 1], F32, tag="t2")
              red_sb = wpool.tile([C, 2], F32, tag="red")
              bounce_in = dpool.tile([C, 2], F32)
              bounce_out = dpool.tile([C, 2], F32)
              for i in range(KN // CHUNK):
                nc.vector.bn_stats(out=bnst[:, i * 6:(i + 1) * 6],
                                   in_=z1[:, i * CHUNK:(i + 1) * CHUNK])
              nc.vector.bn_aggr(out=aggr[:], in_=bnst[:])
              _stats_to_scales(nc, aggr, g1_sb, be1_sb, sc, s1, t1,
                               bounce_in, bounce_out, red_sb)

              # ---- conv2 streamed (fp32r matmul); bn_stats + running max
              # over k read straight from PSUM ----
              m2 = bpool.tile([C, N], F32, tag="m2")
              bnst2 = wpool.tile([C, 36 * 6], F32, tag="bnst")
              aggr2 = wpool.tile([C, 2], F32, tag="aggr")
              bounce_in2 = dpool.tile([C, 2], F32)
              bounce_out2 = dpool.tile([C, 2], F32)
              for i in range(KN // CHUNK):
                c0 = i * CHUNK
                hch = wpool.tile([C, CHUNK], F32R, tag="hch")
                nc.scalar.activation(out=hch[:], in_=z1[:, c0:c0 + CHUNK],
                                     func=AF.Relu, bias=t1[:, 0:1],
                                     scale=s1[:, 0:1])
                z2_ps = z2pool.tile([C, CHUNK], F32, tag="zps2")
                nc.tensor.matmul(out=z2_ps[:], lhsT=w2t[:], rhs=hch[:],
                                 start=True, stop=True)
                nc.vector.bn_stats(out=bnst2[:, i * 6:(i + 1) * 6],
                                   in_=z2_ps[:])
                # fold running max over k (CHUNK divides N: no k straddling)
                m0 = c0 % N
                if c0 < N:
                    nc.vector.tensor_copy(out=m2[:, m0:m0 + CHUNK],
                                          in_=z2_ps[:])
                else:
                    nc.vector.tensor_tensor(out=m2[:, m0:m0 + CHUNK],
                                            in0=m2[:, m0:m0 + CHUNK],
                                            in1=z2_ps[:],
                                            op=AT.max)
              nc.vector.bn_aggr(out=aggr2[:], in_=bnst2[:])
              _stats_to_scales(nc, aggr2, g2_sb, be2_sb, sc, s2, t2,
                               bounce_in2, bounce_out2, red_sb)

              # ---- final relu(s2*m2 + t2) ----
              osb = bpool.tile([C, N], F32, tag="osb")
              nc.scalar.activation(out=osb[:], in_=m2[:], func=AF.Relu,
                                   bias=t2[:, 0:1], scale=s2[:, 0:1])
              nc.sync.dma_start(out=out_d[:, :], in_=osb[:])

    nc.compile()
    return nc


def make_in_maps(inputs):
    x = np.ascontiguousarray(
        np.asarray(inputs["features"], np.float32).reshape(B, C, N))
    w1 = np.asarray(inputs["w1"], np.float32)
    w2 = np.asarray(inputs["w2"], np.float32)
    wc, wd = w1[:, :C], w1[:, C:]
    assert np.all(np.asarray(inputs["g2"], np.float32) > 0), \
        "fused max-then-relu path requires g2 > 0"
    import ml_dtypes
    pwin = np.zeros((C, 1152), ml_dtypes.bfloat16)
    pwin[np.arange(C), 512 + np.arange(C)] = ml_dtypes.bfloat16(NEG)
    const = {
        "negib": np.eye(C, dtype=ml_dtypes.bfloat16),
        "negi": np.eye(C, dtype=np.float32),
        "pwin": pwin,
        "wct": np.ascontiguousarray(wc.T),
        "wst": np.ascontiguousarray((wc + wd).T),
        "w2t": np.ascontiguousarray(w2.T),
        "ones": np.ones((C, 1), np.float32),
        "g1": np.asarray(inputs["g1"], np.float32).reshape(C, 1),
        "be1": np.asarray(inputs["be1"], np.float32).reshape(C, 1),
        "g2": np.asarray(inputs["g2"], np.float32).reshape(C, 1),
        "be2": np.asarray(inputs["be2"], np.float32).reshape(C, 1),
    }
    return [{**const, "x": x[b],
             "btbl": np.ascontiguousarray(-(x[b].T @ wd.T))}
            for b in range(B)]


_NC_CACHE = {}


def kernel(**inputs) -> np.ndarray:
    in_maps = make_in_maps(inputs)
    if "nc" not in _NC_CACHE:
        _NC_CACHE["nc"] = build_nc()
    nc = _NC_CACHE["nc"]
    res = bass_utils.run_bass_kernel_spmd(nc, in_maps,
                                          core_ids=list(range(NCORES)))
    out = np.stack([res.results[b]["out"] for b in range(B)])  # [B,C,N]
    return out[..., None].astype(np.float32)



# revision 5
# speedup vs baseline: 148.0060x; 1.0308x over previous
"""DGCNN edge-conv block (knn9 + 2x conv1x1/BN/relu + max over k) on 8 TRN2 cores.

Sharding: data-parallel over batch B=8 (one sample per NeuronCore).
Cross-core traffic: two tiny AllReduces ([128,2] f32 sum/sumsq) for the
train-mode batchnorm statistics, which span the whole batch.

Per-core pipeline (all on-chip, layout = channels on partitions):
  0. A dummy AllReduce is issued first thing: per-core launch dispatch
     arrives with multi-100us skew through the PJRT tunnel, and the first
     collective in the program absorbs that skew as rendezvous wait.  The
     dummy pays it concurrently with the KNN/conv1 loop, so the two real
     stats AllReduces later only pay their small marginal latency.
  1. KNN: s = x^T x - xx/2 per 128-row tile (fp32 matmuls into two
     double-buffered [128,1000] PSUM half-tiles; the -1e30 self-mask is a
     bf16 identity-x-window matmul accumulated into the diagonal chunk; DVE
     subtracts the xx/2 row and extracts top-8 via max8 + max_index).  The
     per-row constant xx_m does not change each row's ordering and is
     dropped; exact-f32 distances are required (the 9/10-boundary min gap
     on this data is 3.1e-5, so no fp32r/bf16 for the distance matmuls).
  2. conv1 z1[c,(k,n)] interleaved per row-tile with the KNN: one indirect
     row gather per (tile,k) from the host-precomputed -B^T = -(x^T Wd^T)
     f32 table; each k slot seeds A0 = (Wc+Wd)@x into PSUM on PE
     (start=True, stop=False), accumulates the is_transpose of the
     gathered -Wd@x_nbr rows on top (start=False), and ACT drains the
     finished slot to z1.  This keeps the per-slot combine+drain off DVE
     (the bottleneck engine at ~209us/iter busy); PE has slack for the
     8x/tile wst@x re-seed.  The self slot is just Wc@x.
     Biases b1/b2 cancel in train-mode BN and never touch the device.
     (A single multi-offset gather per tile does NOT work: the DMA only
     honours the first offset per partition row and streams consecutive
     table rows after it.  gpsimd.dma_gather computes wrong results on
     real HW despite exact simulator results — do not use it.)
  3. BN1: DVE bn_stats/bn_aggr -> AllReduce(sum,sumsq) -> fused
     relu(s1*z1+t1) on ACT (per-partition scale/bias).
  4. conv2 streamed in 450-wide chunks over the n-major z1 (z1[c,
     n*K+k]; equal chunk sizes keep bn_aggr's equal-count aggregation
     exact): ACT relu -> PE fp32r matmul (adds ~1.7e-4 max-rel on the
     final output vs the 2e-2 gate); the K values per position are
     adjacent, so max-over-k is one innermost tensor_reduce per chunk
     straight from PSUM (4.5% faster than the running-fold layout,
     same-session HW A/B); bn_stats also from PSUM -> AllReduce ->
     final relu(s2*m+t2) (g2>0, host-checked).
"""

import os
import numpy as np

import concourse.bass as bass
import concourse.mybir as mybir
import concourse.bacc as bacc
import concourse.tile as tile
from concourse import bass_utils

F32 = mybir.dt.float32
F32R = mybir.dt.float32r
BF16 = mybir.dt.bfloat16
U32 = mybir.dt.uint32

B, C, N, K = 8, 128, 2000, 9
NCORES = 8
KN = K * N                  # 18000
NL = float(KN)              # local BN count
NG = float(B * KN)          # global BN count
EPS = 1e-5
NEG = -1.0e30
# conv2 / stats chunks over flat n-major z1 (z1[c, n*K+k]): 450 = 50
# positions x K, equal-sized so bn_aggr's equal-count aggregation holds,
# and 450 fp32 fits one PSUM bank.  40 chunks x 450 = 18000.
CHUNKS = [(i * 450, 450) for i in range(40)]

ROW_TILES = [(i * 128, min(128, N - i * 128)) for i in range((N + 127) // 128)]


def _stats_to_scales(nc, aggr, gamma, beta, sc, out_s, out_t, bounce_in,
                     bounce_out, red_sb):
    """aggr [128,2]=(mean,var) local -> AllReduce(sum,sumsq) -> s,t [128,1]."""
    AT = mybir.AluOpType
    # pack local (sum, sumsq) = (mean*NL, (var+mean^2)*NL)
    nc.vector.tensor_scalar(out=sc[:, 0:1], in0=aggr[:, 0:1], scalar1=NL,
                            scalar2=None, op0=AT.mult)
    nc.vector.tensor_tensor(out=sc[:, 1:2], in0=aggr[:, 0:1],
                            in1=aggr[:, 0:1], op=AT.mult)
    nc.vector.tensor_tensor(out=sc[:, 1:2], in0=sc[:, 1:2], in1=aggr[:, 1:2],
                            op=AT.add)
    nc.vector.tensor_scalar(out=sc[:, 1:2], in0=sc[:, 1:2], scalar1=NL,
                            scalar2=None, op0=AT.mult)
    if os.environ.get("NN_DS_SKIP_COLL"):
        nc.vector.tensor_scalar(out=red_sb[:], in0=sc[:, 0:2],
                                scalar1=float(NCORES), scalar2=None,
                                op0=AT.mult)
    else:
        nc.sync.dma_start(out=bounce_in[:], in_=sc[:, 0:2])
        nc.gpsimd.collective_compute(
            "AllReduce", AT.add, replica_groups=[list(range(NCORES))],
            ins=[bounce_in[:].opt()], outs=[bounce_out[:].opt()])
        nc.sync.dma_start(out=red_sb[:], in_=bounce_out[:])
    # gmean = gsum/NG ; gvar = gsumsq/NG - gmean^2
    gmean = sc[:, 2:3]
    gvar = sc[:, 3:4]
    nc.vector.tensor_scalar(out=gmean, in0=red_sb[:, 0:1], scalar1=1.0 / NG,
                            scalar2=None, op0=AT.mult)
    nc.vector.tensor_scalar(out=gvar, in0=red_sb[:, 1:2], scalar1=1.0 / NG,
                            scalar2=None, op0=AT.mult)
    nc.vector.tensor_tensor(out=sc[:, 4:5], in0=gmean, in1=gmean, op=AT.mult)
    nc.vector.tensor_tensor(out=gvar, in0=gvar, in1=sc[:, 4:5], op=AT.subtract)
    # s = gamma * rsqrt(gvar+eps) ; t = beta - s*gmean
    nc.vector.tensor_scalar(out=gvar, in0=gvar, scalar1=EPS, scalar2=None,
                            op0=AT.add)
    nc.scalar.activation(out=sc[:, 5:6], in_=gvar,
                         func=mybir.ActivationFunctionType.Sqrt)
    nc.vector.reciprocal(out=sc[:, 6:7], in_=sc[:, 5:6])
    nc.vector.tensor_tensor(out=out_s[:], in0=sc[:, 6:7], in1=gamma[:],
                            op=AT.mult)
    nc.vector.tensor_tensor(out=sc[:, 7:8], in0=out_s[:], in1=gmean,
                            op=AT.mult)
    nc.vector.tensor_tensor(out=out_t[:], in0=beta[:], in1=sc[:, 7:8],
                            op=AT.subtract)


def build_nc(num_devices=NCORES, reps=1):
    """reps>1 repeats the whole per-core pipeline on device (same inputs,
    same output, identical work each pass) so test.py can measure the
    per-iteration device time as the slope of T(reps) — the fixed ~80ms
    PJRT-tunnel launch overhead cancels in the difference.  The shipped
    kernel() path uses reps=1 and is unchanged."""
    nc = bacc.Bacc("TRN2", target_bir_lowering=False, debug=False,
                   num_devices=num_devices)
    AT = mybir.AluOpType
    AF = mybir.ActivationFunctionType

    x_d = nc.dram_tensor("x", [C, N], F32, kind="ExternalInput").ap()
    wct_d = nc.dram_tensor("wct", [C, C], F32, kind="ExternalInput").ap()
    wst_d = nc.dram_tensor("wst", [C, C], F32, kind="ExternalInput").ap()
    w2t_d = nc.dram_tensor("w2t", [C, C], F32R, kind="ExternalInput").ap()
    negib_d = nc.dram_tensor("negib", [C, C], BF16, kind="ExternalInput").ap()
    negi_d = nc.dram_tensor("negi", [C, C], F32, kind="ExternalInput").ap()
    pwin_d = nc.dram_tensor("pwin", [C, 1152], BF16, kind="ExternalInput").ap()
    ones_d = nc.dram_tensor("ones", [C, 1], F32, kind="ExternalInput").ap()
    g1_d = nc.dram_tensor("g1", [C, 1], F32, kind="ExternalInput").ap()
    be1_d = nc.dram_tensor("be1", [C, 1], F32, kind="ExternalInput").ap()
    g2_d = nc.dram_tensor("g2", [C, 1], F32, kind="ExternalInput").ap()
    be2_d = nc.dram_tensor("be2", [C, 1], F32, kind="ExternalInput").ap()
    out_d = nc.dram_tensor("out", [C, N], F32, kind="ExternalOutput").ap()
    bt_d = nc.dram_tensor("btbl", [N, C], F32, kind="ExternalInput").ap()

    with tile.TileContext(nc) as tc:
        with (
            tc.tile_pool(name="const", bufs=1) as cpool,
            tc.tile_pool(name="big", bufs=1) as bpool,
            tc.tile_pool(name="ps", bufs=2, space="PSUM") as pspool,
            tc.tile_pool(name="zp", bufs=2, space="PSUM") as zpool,
            tc.tile_pool(name="zp2", bufs=2, space="PSUM") as z2pool,
            tc.tile_pool(name="work", bufs=3) as wpool,
            tc.tile_pool(name="dram", bufs=8, space="DRAM") as dpool,
        ):
            def cload(dram, shape, tag, dt=F32):
                t = cpool.tile(shape, dt, tag=tag)
                nc.sync.dma_start(out=t[:], in_=dram)
                return t

            ones_sb = cload(ones_d, [C, 1], "ones_sb")
            x_sb = cload(x_d, [C, N], "x_sb")
            wct = cload(wct_d, [C, C], "wct")
            wst = cload(wst_d, [C, C], "wst")
            w2t = cload(w2t_d, [C, C], "w2t", F32R)
            negib = cload(negib_d, [C, C], "negib", BF16)
            negi = cload(negi_d, [C, C], "negi")
            pwin = cload(pwin_d, [C, 1152], "pwin", BF16)
            g1_sb = cload(g1_d, [C, 1], "g1_sb")
            be1_sb = cload(be1_d, [C, 1], "be1_sb")
            g2_sb = cload(g2_d, [C, 1], "g2_sb")
            be2_sb = cload(be2_d, [C, 1], "be2_sb")

            # ---- dummy collective: absorb cross-core dispatch skew now,
            # concurrently with all the compute below, so the two real
            # stats AllReduces only pay marginal latency.
            if not os.environ.get("NN_DS_SKIP_COLL") and \
               not os.environ.get("NN_DS_NO_DUMMY_CC"):
                dummy_in = dpool.tile([C, 1], F32)
                dummy_out = dpool.tile([C, 1], F32)
                nc.sync.dma_start(out=dummy_in[:], in_=ones_sb[:])
                nc.gpsimd.collective_compute(
                    "AllReduce", AT.add, replica_groups=[list(range(NCORES))],
                    ins=[dummy_in[:].opt()], outs=[dummy_out[:].opt()])

            # bpool tiles are tagged so that with reps>1 every pass reuses
            # the same SBUF slots (WAR deps serialize passes; SBUF use is
            # independent of reps).
            for _rep in range(reps):
              # ---- xxh = 0.5 * sum_c x^2, broadcast to all partitions ----
              xsq = bpool.tile([C, N], F32, tag="xsq")
              nc.vector.tensor_tensor(out=xsq[:], in0=x_sb[:], in1=x_sb[:],
                                      op=AT.mult)
              xxh_row = bpool.tile([1, N], F32, tag="xxh_row")
              for h0 in (0, N // 2):
                xx_ps = pspool.tile([1, N // 2], F32, tag="pd")
                for c0, cw in ((0, 512), (512, 488)):
                    nc.tensor.matmul(out=xx_ps[:, c0:c0 + cw], lhsT=ones_sb[:],
                                     rhs=xsq[:, h0 + c0:h0 + c0 + cw],
                                     start=True, stop=True)
                nc.scalar.mul(out=xxh_row[:, h0:h0 + N // 2], in_=xx_ps[:],
                              mul=0.5)
              xxh = bpool.tile([C, N], F32, tag="xxh")
              nc.gpsimd.partition_broadcast(xxh[:], xxh_row[:])

              # ---- KNN + conv1, interleaved per row-tile so the z1 work
              # (PE seeds/transposes, gather DMAs, ACT copies) overlaps the
              # DVE top-k of later tiles.  pd uses a single 4-bank PSUM slot;
              # the small z tiles triple-buffer in their own banks.
              idx_all = bpool.tile([128, 8 * len(ROW_TILES)], U32,
                                   tag="idx_all")
              z1 = bpool.tile([C, KN], F32, tag="z1")
              # n-major view: k values per position adjacent
              z1v = z1.rearrange("c (n k) -> c n k", k=K)
              for ti, (n0, nr) in enumerate(ROW_TILES):
                pd_sb = wpool.tile([128, N], F32, tag="pdsb")
                for h0 in (0, N // 2):
                    pd_ps = pspool.tile([128, N // 2], F32, tag="pd")
                    for c0, cw in ((0, 512), (512, 488)):
                        a0c = h0 + c0
                        hit = n0 < a0c + cw and n0 + nr > a0c
                        nc.tensor.matmul(out=pd_ps[:nr, c0:c0 + cw],
                                         lhsT=x_sb[:, n0:n0 + nr],
                                         rhs=x_sb[:, a0c:a0c + cw],
                                         start=True, stop=not hit)
                        if hit:
                            woff = 512 - (n0 - a0c)
                            nc.tensor.matmul(out=pd_ps[:nr, c0:c0 + cw],
                                             lhsT=negib[:, :nr],
                                             rhs=pwin[:, woff:woff + cw],
                                             start=False, stop=True)
                    nc.vector.tensor_tensor(out=pd_sb[:nr, h0:h0 + N // 2],
                                            in0=pd_ps[:nr, :],
                                            in1=xxh[:nr, h0:h0 + N // 2],
                                            op=AT.subtract)
                mx8 = wpool.tile([128, 8], F32, tag="mx8")
                nc.vector.max(out=mx8[:nr, :], in_=pd_sb[:nr, :])
                nc.vector.max_index(out=idx_all[:nr, ti * 8:ti * 8 + 8],
                                    in_max=mx8[:nr, :], in_values=pd_sb[:nr, :])
                # conv1 z1 for this tile; self slot (k=0): Wc @ x.
                # A0 = (Wc+Wd)@x is computed ONCE per tile into SBUF; each
                # k slot is then a standalone PE transpose of the gathered
                # -Wd@x_nbr rows plus a DVE add (A0 + -g^T) straight into
                # z1 — saves 8 redundant wst@x re-seed matmuls per tile
                # and moves the PSUM drain off the ACT engine.
                z_ps = zpool.tile([128, 128], F32, tag="zps")
                nc.tensor.matmul(out=z_ps[:, :nr], lhsT=wct[:],
                                 rhs=x_sb[:, n0:n0 + nr], start=True, stop=True)
                nc.scalar.copy(out=z1v[:, n0:n0 + nr, 0], in_=z_ps[:, :nr])
                # Each k slot: PE seeds A0=(Wc+Wd)@x into PSUM, the
                # gathered -Wd@x_nbr rows accumulate on top via the
                # is_transpose matmul (start=False), and ACT drains to
                # z1.  This takes the per-slot combine+drain off DVE
                # (the bottleneck engine at ~209us/iter) at the cost of
                # re-seeding wst@x on PE, which has slack.
                for k in range(8):
                    g_sb = wpool.tile([128, C], F32, tag="gath")
                    nc.gpsimd.indirect_dma_start(
                        out=g_sb[:nr, :], out_offset=None, in_=bt_d[:, :],
                        in_offset=bass.IndirectOffsetOnAxis(
                            ap=idx_all[:nr, ti * 8 + k:ti * 8 + k + 1],
                            axis=0))
                    zt_ps = zpool.tile([128, 128], F32, tag="zps")
                    nc.tensor.matmul(out=zt_ps[:, :nr], lhsT=wst[:],
                                     rhs=x_sb[:, n0:n0 + nr],
                                     start=True, stop=False)
                    # -g^T (btbl rows are negated on host)
                    nc.tensor.matmul(out=zt_ps[:, :nr],
                                     lhsT=g_sb[:nr, :],
                                     rhs=negi[:nr, :nr], is_transpose=True,
                                     start=False, stop=True)
                    nc.scalar.copy(out=z1v[:, n0:n0 + nr, k + 1],
                                   in_=zt_ps[:, :nr])

              # ---- BN1 stats + allreduce -> s1,t1 ----
              sc = bpool.tile([C, 8], F32, tag="sc")
              bnst = wpool.tile([C, 40 * 6], F32, tag="bnst")
              aggr = wpool.tile([C, 2], F32, tag="aggr")
              s1 = bpool.tile([C, 1], F32, tag="s1")
              t1 = bpool.tile([C, 1], F32, tag="t1")
              s2 = bpool.tile([C, 1], F32, tag="s2")
              t2 = bpool.tile([C, 1], F32, tag="t2")
              red_sb = wpool.tile([C, 2], F32, tag="red")
              bounce_in = dpool.tile([C, 2], F32)
              bounce_out = dpool.tile([C, 2], F32)
              for i, (c0, cw) in enumerate(CHUNKS):
                nc.vector.bn_stats(out=bnst[:, i * 6:(i + 1) * 6],
                                   in_=z1[:, c0:c0 + cw])
              nc.vector.bn_aggr(out=aggr[:], in_=bnst[:])
              _stats_to_scales(nc, aggr, g1_sb, be1_sb, sc, s1, t1,
                               bounce_in, bounce_out, red_sb)

              # ---- conv2 streamed (fp32r matmul); bn_stats + running max
              # over k read straight from PSUM ----
              m2 = bpool.tile([C, N], F32, tag="m2")
              bnst2 = wpool.tile([C, 40 * 6], F32, tag="bnst")
              aggr2 = wpool.tile([C, 2], F32, tag="aggr")
              bounce_in2 = dpool.tile([C, 2], F32)
              bounce_out2 = dpool.tile([C, 2], F32)
              for i, (c0, cw) in enumerate(CHUNKS):
                np_ = cw // K
                p0 = c0 // K
                hch = wpool.tile([C, 450], F32R, tag="hch")
                nc.scalar.activation(out=hch[:, :cw], in_=z1[:, c0:c0 + cw],
                                     func=AF.Relu, bias=t1[:, 0:1],
                                     scale=s1[:, 0:1])
                z2_ps = z2pool.tile([C, 450], F32, tag="zps2")
                nc.tensor.matmul(out=z2_ps[:, :cw], lhsT=w2t[:],
                                 rhs=hch[:, :cw], start=True, stop=True)
                nc.vector.bn_stats(out=bnst2[:, i * 6:(i + 1) * 6],
                                   in_=z2_ps[:, :cw])
                # n-major: max over the K adjacent values per position,
                # one innermost reduce per chunk (replaces the running
                # fold and the k=0 seed copies)
                nc.vector.tensor_reduce(
                    out=m2[:, p0:p0 + np_],
                    in_=z2_ps[:, :cw].rearrange("c (n k) -> c n k", k=K),
                    axis=mybir.AxisListType.X, op=AT.max)
              nc.vector.bn_aggr(out=aggr2[:], in_=bnst2[:])
              _stats_to_scales(nc, aggr2, g2_sb, be2_sb, sc, s2, t2,
                               bounce_in2, bounce_out2, red_sb)

              # ---- final relu(s2*m2 + t2) ----
              osb = bpool.tile([C, N], F32, tag="osb")
              nc.scalar.activation(out=osb[:], in_=m2[:], func=AF.Relu,
                                   bias=t2[:, 0:1], scale=s2[:, 0:1])
              nc.sync.dma_start(out=out_d[:, :], in_=osb[:])

    nc.compile()
    return nc


def make_in_maps(inputs):
    x = np.ascontiguousarray(
        np.asarray(inputs["features"], np.float32).reshape(B, C, N))
    w1 = np.asarray(inputs["w1"], np.float32)
    w2 = np.asarray(inputs["w2"], np.float32)
    wc, wd = w1[:, :C], w1[:, C:]
    assert np.all(np.asarray(inputs["g2"], np.float32) > 0), \
        "fused max-then-relu path requires g2 > 0"
    import ml_dtypes
    pwin = np.zeros((C, 1152), ml_dtypes.bfloat16)
    pwin[np.arange(C), 512 + np.arange(C)] = ml_dtypes.bfloat16(NEG)
    const = {
        "negib": np.eye(C, dtype=ml_dtypes.bfloat16),
        "negi": np.eye(C, dtype=np.float32),
        "pwin": pwin,
        "wct": np.ascontiguousarray(wc.T),
        "wst": np.ascontiguousarray((wc + wd).T),
        "w2t": np.ascontiguousarray(w2.T),
        "ones": np.ones((C, 1), np.float32),
        "g1": np.asarray(inputs["g1"], np.float32).reshape(C, 1),
        "be1": np.asarray(inputs["be1"], np.float32).reshape(C, 1),
        "g2": np.asarray(inputs["g2"], np.float32).reshape(C, 1),
        "be2": np.asarray(inputs["be2"], np.float32).reshape(C, 1),
    }
    return [{**const, "x": x[b],
             "btbl": np.ascontiguousarray(-(x[b].T @ wd.T))}
            for b in range(B)]


_NC_CACHE = {}


def kernel(**inputs) -> np.ndarray:
    in_maps = make_in_maps(inputs)
    if "nc" not in _NC_CACHE:
        _NC_CACHE["nc"] = build_nc()
    nc = _NC_CACHE["nc"]
    res = bass_utils.run_bass_kernel_spmd(nc, in_maps,
                                          core_ids=list(range(NCORES)))
    out = np.stack([res.results[b]["out"] for b in range(B)])  # [B,C,N]
    return out[..., None].astype(np.float32)

